# revision 12
# baseline (speedup 1.0000x reference)
"""CoaT factorized-attention + CRPE block on 8 Trainium2 NeuronCores.

Sharding: pure data-parallel over batch B=32 -> 4 images per core.
Per-core layouts (all chosen so NO on-device transposes are needed):
  xt      [128, 4*NP]      feature-major input, 4 channel-tiles packed along
                           the free dim (host pre-transposes); fp16
  q       [128, 787]       feature-major, stored shifted +1 col so the image
                           tokens (1..784) sit at even fp16 offsets for DVE
  k       [T, C]           token-major fp16 (GEMM-KV: lhsT=xt tiles)
  v       [T, 4*(128+2)]   token-major fp16, per-head-pair chunk padded with
                           two 8.0 columns so the kv matmul also produces the
                           softmax denominator scaled by 1/SCALE
  kv      [c, d] per head-pair, block-diag packed 128x128, fp16; off-diag
                           blocks zeroed once, diagonals rescaled on DVE
  fa      PSUM; CRPE added into PSUM by DVE, single scalar copy to attn
  conv    feature-major on a zero-padded 34x34 fp16 image buffer; per-image
          per-channel-tile tap split across PE (fp16 diag matmuls), DVE and
          ScalarE (multiplies), tuned so all three engines stay balanced;
          the last image leans on PE (nothing else fills the pipeline tail)
  proj    out[T, 512] token-major (lhsT=attn fp16, rhs=proj_w.T fp16)
Softmax over tokens without max-subtraction (values are O(1)): ek=exp(k) on
ScalarE; 1/den and the 1/sqrt(Ch) scale fold into the kv rescale.
PSUM discipline: 4 independent tags (kv-gemm / q+v2+hp / fa+proj / conv),
every matmul group exactly 1 bank, so image b+1's GEMMs never wait on
image b's late-phase consumers. 8 banks exactly.
DMA discipline: one packed DMA per weight blob, one per image input, one per
image output (out padded to 896 rows); image-0 input issued before the bulky
conv-diag weight blob so PE starts within ~6us.
"""

import numpy as np

import concourse.bass as bass
import concourse.bacc as bacc
import concourse.mybir as mybir
import concourse.tile as tile
from concourse.bass_utils import run_bass_kernel_spmd

F32 = mybir.dt.float32
F16 = mybir.dt.float16

# taps on PE per channel-tile (bias always rides with PE when >0):
# mid-pipeline images keep PE lean; the last image fills the tail with PE.
CWD_TAPS = [9, 25, 49, 16]           # diag tiles built per ct
PE_TAPS_MID = [0, 25, 30, 0]
PE_TAPS_LAST = [9, 25, 49, 16]
DVM = 3                              # every DVM-th non-PE tap multiply -> DVE; rest ScalarE

NCORES = 8
B, N, C = 32, 785, 512
BL = B // NCORES            # 4 images per core
H = W = 28
HW = H * W                  # 784, N = 1 + HW
NH = 8                      # heads
CH = C // NH                # 64
SCALE = CH ** -0.5          # 1/8
PADW = 34                   # 28 + 2*3 (pad 3 covers 3x3/5x5/7x7 uniformly)
NP = 786                    # token columns padded even
NT = 7                      # token tiles: 6*128 + 17
TSIZES = [128, 128, 128, 128, 128, 128, 17]
TOFFS = [0, 128, 256, 384, 512, 640, 768]
NPAD = 896                  # 7*128, padded output rows per image
GS = 505                    # token split: 504 pixels = 18 image rows
VW = 130                    # v chunk width per head-pair: 128 + 2 ones cols

CT_TAPS = [(3, 1), (5, 2), (7, 3), (7, 3)]
N_DIAG = sum(t + 1 for t in CWD_TAPS)


def _tap_base(k):
    return 3 - (k // 2)


def build_conv_weights(w3, b3, w5, b5, w7, b7):
    """Per channel-tile tap weights [4][128, 49] and biases [128, 4]."""
    w3 = w3.reshape(128, 9).astype(np.float32)
    w5 = w5.reshape(192, 25).astype(np.float32)
    w7 = w7.reshape(192, 49).astype(np.float32)
    cw = np.zeros((4, 128, 49), np.float32)
    cw[0, :, :9] = w3
    cw[1, :, :25] = w5[:128]
    emb = np.zeros((64, 7, 7), np.float32)
    emb[:, 1:6, 1:6] = w5[128:192].reshape(64, 5, 5)
    cw[2, :64] = emb.reshape(64, 49)
    cw[2, 64:] = w7[:64]
    cw[3] = w7[64:192]
    cb = np.zeros((128, 4), np.float32)
    cb[:, 0] = b3
    cb[:, 1] = b5[:128]
    cb[:64, 2] = b5[128:192]
    cb[64:, 2] = b7[:64]
    cb[:, 3] = b7[64:192]
    return cw, cb


def build_cwd(cw, cb):
    """Packed diag tap weights [128, N_DIAG*128] fp16 (bias diag first)."""
    ar = np.arange(128)
    diags = []
    for ct in range(4):
        d = np.zeros((128, 128), np.float16)
        d[ar, ar] = cb[:, ct].astype(np.float16)
        diags.append(d)
        for ti in range(CWD_TAPS[ct]):
            d = np.zeros((128, 128), np.float16)
            d[ar, ar] = cw[ct][:, ti].astype(np.float16)
            diags.append(d)
    return np.stack(diags).transpose(1, 0, 2).reshape(128, -1)


def pack_rows(wT, nchunk):
    """[nchunk*128, X] -> [128, nchunk*X] partition-major packing."""
    X = wT.shape[1]
    return np.ascontiguousarray(
        wT.reshape(nchunk, 128, X).transpose(1, 0, 2).reshape(128, nchunk * X))


def build_nc(has_qkv_bias):
    nc = bacc.Bacc()

    def mm(out, lhsT, rhs, **kw):
        nc.tensor.matmul(out, lhsT, rhs, **kw)

    xt_d = nc.dram_tensor("xt", [BL, 128, 4 * NP], F16, kind="ExternalInput")
    wq_d = nc.dram_tensor("wq", [128, 4 * C], F16, kind="ExternalInput")
    wkv_d = nc.dram_tensor("wkv", [128, 4 * 2 * C], F16, kind="ExternalInput")
    wv_d = nc.dram_tensor("wv", [128, 4 * C], F16, kind="ExternalInput")
    pw_d = nc.dram_tensor("pw", [128, 4 * C], F16, kind="ExternalInput")
    cw_d = nc.dram_tensor("cw", [128, 4 * 49], F32, kind="ExternalInput")
    cb_d = nc.dram_tensor("cb", [128, 4], F32, kind="ExternalInput")
    cwd_d = nc.dram_tensor("cwd", [128, N_DIAG * 128], F16, kind="ExternalInput")
    if has_qkv_bias:
        bq_d = nc.dram_tensor("bq", [128, 4], F32, kind="ExternalInput")
        bv_d = nc.dram_tensor("bv", [128, 4], F32, kind="ExternalInput")
        bkv_d = nc.dram_tensor("bkv", [1, 2 * C], F16, kind="ExternalInput")
    out_d = nc.dram_tensor("out", [BL, NPAD, C], F16, kind="ExternalOutput")

    with tile.TileContext(nc) as tc:
        with (
            tc.tile_pool(name="const", bufs=1) as cpool,
            tc.tile_pool(name="xt", bufs=2) as xtpool,
            tc.tile_pool(name="ek", bufs=14) as ekpool,
            tc.tile_pool(name="vt", bufs=14) as vtpool,
            tc.tile_pool(name="qf", bufs=8) as qpool,
            tc.tile_pool(name="vp", bufs=2) as vppool,
            tc.tile_pool(name="ca", bufs=2) as capool,
            tc.tile_pool(name="ev", bufs=2) as evpool,
            tc.tile_pool(name="at", bufs=8) as atpool,
            tc.tile_pool(name="sm", bufs=8) as smpool,
            tc.tile_pool(name="ob", bufs=2) as obpool,
            tc.tile_pool(name="pskv", bufs=2, space="PSUM") as kvpool,
            tc.tile_pool(name="psq", bufs=2, space="PSUM") as qppool,
            tc.tile_pool(name="psfa", bufs=2, space="PSUM") as fapool,
            tc.tile_pool(name="psc", bufs=2, space="PSUM") as pscpool,
        ):
            # ---- image-0 input DMA first: PE can start within ~6us ----
            xt_bufs = []
            for i in range(2):
                t = xtpool.tile([128, 4 * NP], F16, tag="xt", name=f"xtb{i}")
                xt_bufs.append(t)
            nc.sync.dma_start(xt_bufs[0][:], xt_d[0])

            # ---- constants: one packed DMA per blob ----
            wq_t = cpool.tile([128, 4 * C], F16, tag="wq")
            wkv_t = cpool.tile([128, 8 * C], F16, tag="wkv")
            wv_t = cpool.tile([128, 4 * C], F16, tag="wv")
            pw_t = cpool.tile([128, 4 * C], F16, tag="pw")
            cw_t = cpool.tile([128, 4 * 49], F32, tag="cw")
            cb_t = cpool.tile([128, 4], F32, tag="cb")
            cwd_t = cpool.tile([128, N_DIAG * 128], F16, tag="cwd")
            nc.sync.dma_start(wkv_t[:], wkv_d[:])
            nc.sync.dma_start(wq_t[:], wq_d[:])
            nc.sync.dma_start(wv_t[:], wv_d[:])
            nc.sync.dma_start(cw_t[:], cw_d[:])
            nc.sync.dma_start(cb_t[:], cb_d[:])
            nc.sync.dma_start(pw_t[:], pw_d[:])
            nc.sync.dma_start(cwd_t[:], cwd_d[:])

            def wkv_s(kc, cols):
                return wkv_t[:, kc * 2 * C + cols.start:kc * 2 * C + cols.stop]

            def pw_s(kc):
                return pw_t[:, kc * C:(kc + 1) * C]

            def xt_s(xt, kc, cols):
                return xt[:, kc * NP + cols.start:kc * NP + cols.stop]

            def cwd_s(i):
                return cwd_t[:, i * 128:(i + 1) * 128]

            def cw_s(ct, ti):
                return cw_t[:, ct * 49 + ti:ct * 49 + ti + 1]

            onesimg = cpool.tile([128, HW], F16, tag="onesimg")
            nc.gpsimd.memset(onesimg[:], 1.0)
            if has_qkv_bias:
                bq_t = cpool.tile([128, 4], F32, tag="bq")
                bv_t = cpool.tile([128, 4], F32, tag="bv")
                bkv_t = cpool.tile([1, 2 * C], F16, tag="bkv")
                ones_t = cpool.tile([1, 128], F16, tag="ones")
                nc.sync.dma_start(bq_t[:], bq_d[:])
                nc.sync.dma_start(bv_t[:], bv_d[:])
                nc.sync.dma_start(bkv_t[:], bkv_d[:])
                nc.scalar.activation(ones_t[:], bkv_t[:, 0:128],
                    mybir.ActivationFunctionType.Identity, bias=1.0, scale=0.0)

            # ---- v tiles: denominator columns (=8.0) written ONCE ----
            vt_bufs = []
            for i in range(14):
                t = vtpool.tile([128, 4 * VW], F16, tag="vt", name=f"vt{i}")
                nc.gpsimd.memset(
                    t[:].rearrange("p (h c) -> p h c", h=4)[:, :, 128:VW], 8.0)
                vt_bufs.append(t)

            # ---- kv tiles: off-diagonal head blocks zeroed ONCE ----
            kv_bufs = []
            for i in range(8):
                t = smpool.tile([128, 128], F16, tag="kvsb", name=f"kv{i}")
                nc.gpsimd.memset(t[:], 0.0)
                kv_bufs.append(t)

            # ---- padded conv image buffers: memset borders ONCE ----
            vp_bufs, vq_bufs = [], []
            for ct in range(4):
                bs, qs = [], []
                for i in range(2):
                    t = vppool.tile([128, PADW, PADW], F16,
                                    tag=f"vp{ct}", name=f"vp{ct}_{i}")
                    nc.gpsimd.memset(t[:], 0.0)
                    bs.append(t)
                    if PE_TAPS_MID[ct] < CT_TAPS[ct][0] ** 2:
                        tq = vppool.tile([128, PADW, PADW], F16,
                                         tag=f"vq{ct}", name=f"vq{ct}_{i}")
                        nc.gpsimd.memset(tq[:], 0.0)
                        qs.append(tq)
                vp_bufs.append(bs)
                vq_bufs.append(qs)

            ob_bufs = []
            for i in range(2):
                t = obpool.tile([128, NT * C], F16, tag="ob", name=f"ob{i}")
                nc.gpsimd.memset(t[:, (NT - 1) * C:], 0.0)
                ob_bufs.append(t)

            diag_off = {}
            _o = 0
            for _ct in range(4):
                diag_off[_ct] = _o
                _o += CWD_TAPS[_ct] + 1

            for b in range(BL):
                xt_t = xt_bufs[b % 2]
                if b + 1 < BL:
                    nxt = xt_bufs[(b + 1) % 2]
                    nc.sync.dma_start(nxt[:], xt_d[b + 1])
                pe_taps = PE_TAPS_LAST if b == BL - 1 else PE_TAPS_MID

                # ---- GEMM-KV: token-major ek=exp(k) and v (+den cols) ----
                ek_t, v_t = [], []
                for tt in range(NT):
                    m = TSIZES[tt]
                    o = TOFFS[tt]
                    ek = ekpool.tile([128, C], F16, tag="ek")
                    vv = vt_bufs[(b * NT + tt) % 14]
                    for half in range(2):
                        cols = slice(512 * half, 512 * (half + 1))
                        ps = kvpool.tile([128, 512], F32, tag="pskv",
                                         name="pskv")
                        for kc in range(4):
                            mm(
                                ps[:m, :],
                                xt_s(xt_t, kc, slice(o, o + m)),
                                wkv_s(kc, cols),
                                start=(kc == 0),
                                stop=(kc == 3 and not has_qkv_bias),
                            )
                        if has_qkv_bias:
                            mm(ps[:m, :], ones_t[:, :m], bkv_t[:, cols],
                               start=False, stop=True)
                        if half == 0:
                            nc.scalar.activation(
                                ek[:m, :], ps[:m, :],
                                mybir.ActivationFunctionType.Exp)
                        else:
                            nc.scalar.copy(
                                vv[:m].rearrange(
                                    "p (h c) -> p h c", h=4)[:, :, 0:128],
                                ps[:m, :].rearrange(
                                    "p (h c) -> p h c", h=4))
                    ek_t.append(ek)
                    v_t.append(vv)

                # ---- GEMM-Q: feature-major q, stored shifted +1 col ----
                q_t = []
                for mo in range(4):
                    q = qpool.tile([128, NP + 1], F16, tag="qf")
                    for cols in (slice(0, GS), slice(GS, NP)):
                        ps = qppool.tile([128, GS], F32, tag="psq", name="psq")
                        w = cols.stop - cols.start
                        for kc in range(4):
                            mm(
                                ps[:, 0:w],
                                wq_t[:, kc * C + 128 * mo:kc * C + 128 * (mo + 1)],
                                xt_s(xt_t, kc, cols),
                                start=(kc == 0),
                                stop=(kc == 3),
                            )
                        if has_qkv_bias:
                            nc.scalar.activation(
                                q[:, 1 + cols.start:1 + cols.stop], ps[:, 0:w],
                                mybir.ActivationFunctionType.Identity,
                                bias=bq_t[:, mo:mo + 1])
                        else:
                            nc.scalar.copy(
                                q[:, 1 + cols.start:1 + cols.stop], ps[:, 0:w])
                    q_t.append(q)

                # ---- GEMM-V2: feature-major v straight into padded image ----
                vpad_t = [vp_bufs[ct][b % 2] for ct in range(4)]
                vpad1_t = [vq_bufs[ct][b % 2] if vq_bufs[ct] else None
                           for ct in range(4)]
                for ct in range(4):
                    vp = vpad_t[ct]
                    for gi, cols in enumerate((slice(0, GS), slice(GS, NP))):
                        ps = qppool.tile([128, GS], F32, tag="psq", name="psv")
                        w = cols.stop - cols.start
                        for kc in range(4):
                            mm(
                                ps[:, 0:w],
                                wv_t[:, kc * C + 128 * ct:kc * C + 128 * (ct + 1)],
                                xt_s(xt_t, kc, cols),
                                start=(kc == 0),
                                stop=(kc == 3),
                            )
                        # g0 tokens 1..504 -> rows 0..17; g1 505..784 -> 18..27
                        if gi == 0:
                            src = ps[:, 1:GS].rearrange(
                                "p (h w) -> p h w", h=18)
                            dst = vp[:, 3:21, 3:31]
                        else:
                            src = ps[:, 0:280].rearrange(
                                "p (h w) -> p h w", h=10)
                            dst = vp[:, 21:31, 3:31]
                        if has_qkv_bias:
                            nc.scalar.activation(
                                dst, src,
                                mybir.ActivationFunctionType.Identity,
                                bias=bv_t[:, ct:ct + 1])
                        else:
                            nc.scalar.copy(dst, src)
                    if vpad1_t[ct] is not None and pe_taps[ct] < CT_TAPS[ct][0] ** 2:
                        # 1-elem-shifted copy so odd-offset taps stay 4B-aligned
                        vq = vpad1_t[ct]
                        nc.vector.tensor_copy(
                            vq[:].rearrange("p a b -> p (a b)")[:, 0:1154],
                            vp[:].rearrange("p a b -> p (a b)")[:, 1:1155])

                # ---- kv per head-pair (+ den in ones cols); rescale on DVE --
                kv_t = []
                recip = smpool.tile([128, 8], F32, tag="recip", bufs=4)
                for hp in range(4):
                    cs = slice(128 * hp, 128 * (hp + 1))
                    vs = slice(VW * hp, VW * (hp + 1))
                    ps = qppool.tile([128, VW], F32, tag="psq", name="pshp")
                    for tt in range(NT):
                        m = TSIZES[tt]
                        mm(
                            ps[:], ek_t[tt][:m, cs], v_t[tt][:m, vs],
                            start=(tt == 0), stop=(tt == NT - 1))
                    nc.vector.reciprocal(
                        recip[:, 2 * hp:2 * hp + 2], ps[:, 128:VW])
                    kv = kv_bufs[(b * 4 + hp) % 8]
                    nc.vector.tensor_scalar_mul(
                        kv[0:64, 0:64], ps[0:64, 0:64],
                        recip[0:64, 2 * hp:2 * hp + 1])
                    nc.vector.tensor_scalar_mul(
                        kv[64:128, 64:128], ps[64:128, 64:128],
                        recip[64:128, 2 * hp:2 * hp + 1])
                    kv_t.append(kv)

                # ---- conv (CRPE): per-ct PE/DVE/ScalarE tap split ----
                convsrc = [None] * 4
                nmul = 0
                for ct in range(4):
                    k, _p = CT_TAPS[ct]
                    kk = k * k
                    base = _tap_base(k)
                    p = pe_taps[ct]
                    di = diag_off[ct]
                    ev = evpool.tile([128, HW], F16, tag=f"ev{ct}",
                                     name=f"ev{ct}")
                    psc_h = [None, None]
                    if p > 0:
                        for hh in range(2):
                            cols = slice(392 * hh, 392 * (hh + 1))
                            yo = 14 * hh
                            psc = pscpool.tile([128, 512], F32, tag="pscv",
                                               name="pscv")
                            mm(psc[:, 0:392], cwd_s(di), onesimg[:, cols],
                               start=True, stop=False)
                            for ti in range(p):
                                i, j = divmod(ti, k)
                                src = vpad_t[ct][
                                    :, base + i + yo:base + i + yo + 14,
                                    base + j:base + j + W]
                                mm(psc[:, 0:392], cwd_s(di + 1 + ti), src,
                                   start=False, stop=(ti == p - 1))
                            psc_h[hh] = psc
                    if p == kk:
                        for hh in range(2):
                            sl = slice(392 * hh, 392 * (hh + 1))
                            nc.vector.tensor_tensor(
                                ev[:, sl], psc_h[hh][:, 0:392],
                                q_t[ct][:, 2 + sl.start:2 + sl.stop],
                                op=mybir.AluOpType.mult)
                        convsrc[ct] = ev
                        continue

                    # DVE/ScalarE part for taps p..kk-1
                    def tap_src(ti):
                        i, j = divmod(ti, k)
                        if (base + i * PADW + base + j) % 2:
                            return vpad1_t[ct][:, base + i:base + i + H,
                                               base + j - 1:base + j - 1 + W]
                        return vpad_t[ct][:, base + i:base + i + H,
                                          base + j:base + j + W]

                    acc = capool.tile([128, H, W], F16, tag=f"ca{ct}",
                                      name=f"ca{ct}")
                    for n, ti in enumerate(range(p, kk)):
                        src = tap_src(ti)
                        if n == 0 and p == 0:
                            nc.scalar.activation(
                                acc[:], src,
                                mybir.ActivationFunctionType.Identity,
                                bias=cb_t[:, ct:ct + 1], scale=cw_s(ct, ti))
                        elif n == 0:
                            nc.vector.tensor_scalar_mul(
                                acc[:], src, cw_s(ct, ti))
                        else:
                            tmp = capool.tile([128, H, W], F16, tag="tp",
                                              name="tp", bufs=4)
                            if nmul % DVM == DVM - 1:
                                nc.vector.tensor_scalar_mul(
                                    tmp[:], src, cw_s(ct, ti))
                            else:
                                nc.scalar.activation(
                                    tmp[:], src,
                                    mybir.ActivationFunctionType.Identity,
                                    scale=cw_s(ct, ti))
                            nc.vector.tensor_tensor(
                                acc[:], acc[:], tmp[:],
                                op=mybir.AluOpType.add)
                            nmul += 1
                    accf = acc[:].rearrange("p h w -> p (h w)")
                    if p > 0:
                        for hh in range(2):
                            sl = slice(392 * hh, 392 * (hh + 1))
                            nc.vector.tensor_tensor(
                                ev[:, sl], psc_h[hh][:, 0:392], accf[:, sl],
                                op=mybir.AluOpType.add)
                            nc.vector.tensor_tensor(
                                ev[:, sl], ev[:, sl],
                                q_t[ct][:, 2 + sl.start:2 + sl.stop],
                                op=mybir.AluOpType.mult)
                    else:
                        nc.vector.tensor_tensor(
                            ev[:], accf, q_t[ct][:, 2:NP],
                            op=mybir.AluOpType.mult)
                    convsrc[ct] = ev

                # ---- factor-att; CRPE added in PSUM; single copy to attn ----
                attn_t = []
                for hp in range(4):
                    at = atpool.tile([128, N], F16, tag="attn")
                    ev = convsrc[hp]
                    for gi, cols in enumerate((slice(0, GS), slice(GS, NP))):
                        ps = fapool.tile([128, GS], F32, tag="psfa",
                                         name="psfa")
                        w = cols.stop - cols.start
                        mm(ps[:, 0:w], kv_t[hp][:],
                           q_t[hp][:, cols.start + 1:cols.stop + 1],
                           start=True, stop=True)
                        if gi == 0:
                            nc.vector.tensor_tensor(
                                ps[:, 1:GS], ps[:, 1:GS], ev[:, 0:GS - 1],
                                op=mybir.AluOpType.add)
                            nc.scalar.copy(at[:, 0:GS], ps[:, 0:GS])
                        else:
                            nc.vector.tensor_tensor(
                                ps[:, 0:280], ps[:, 0:280], ev[:, GS - 1:HW],
                                op=mybir.AluOpType.add)
                            nc.scalar.copy(at[:, GS:N], ps[:, 0:280])
                    attn_t.append(at)

                # ---- proj: out[t, :] token-major, single padded DMA ----
                ob = ob_bufs[b % 2]
                for tt in range(NT):
                    m = TSIZES[tt]
                    o = TOFFS[tt]
                    ps = fapool.tile([128, C], F32, tag="psfa", name="psout")
                    for kc in range(4):
                        mm(
                            ps[:m, 0:C], attn_t[kc][:, o:o + m], pw_s(kc),
                            start=(kc == 0), stop=(kc == 3))
                    nc.scalar.copy(ob[:m, tt * C:(tt + 1) * C], ps[:m, 0:C])
                nc.sync.dma_start(
                    out_d[b].rearrange("(a p) c -> p a c", p=128),
                    ob[:].rearrange("p (a c) -> p a c", a=NT))

    nc.compile()
    return nc


_NC_CACHE = {}


def _get_nc(has_qkv_bias):
    key = bool(has_qkv_bias)
    if key not in _NC_CACHE:
        _NC_CACHE[key] = build_nc(has_qkv_bias)
    return _NC_CACHE[key]


def prep_shared(qkv_w, proj_w, w3, b3, w5, b5, w7, b7):
    qkv_w = np.asarray(qkv_w, np.float32)
    proj_w = np.asarray(proj_w, np.float32)
    wqT = qkv_w[0:C].T
    wkvT = np.concatenate([qkv_w[C:2 * C].T, qkv_w[2 * C:3 * C].T], axis=1)
    wvT = qkv_w[2 * C:3 * C].T
    pwT = proj_w.T
    cw, cb = build_conv_weights(
        np.asarray(w3, np.float32), np.asarray(b3, np.float32),
        np.asarray(w5, np.float32), np.asarray(b5, np.float32),
        np.asarray(w7, np.float32), np.asarray(b7, np.float32))
    return {
        "wq": pack_rows(wqT, 4).astype(np.float16),
        "wkv": pack_rows(wkvT, 4).astype(np.float16),
        "wv": pack_rows(wvT, 4).astype(np.float16),
        "pw": pack_rows(pwT, 4).astype(np.float16),
        "cw": np.ascontiguousarray(
            cw.transpose(1, 0, 2).reshape(128, 4 * 49)),
        "cb": cb,
        "cwd": build_cwd(cw, cb),
    }


def prep_xt(xs):
    """[nb, N, C] f32 -> [nb, 128, 4*NP] f16 packed feature-major."""
    nb = xs.shape[0]
    xt = np.zeros((nb, 128, 4 * NP), np.float16)
    xsT = xs.transpose(0, 2, 1)  # [nb, C, N]
    for ct in range(4):
        xt[:, :, ct * NP:ct * NP + N] = xsT[:, ct * 128:(ct + 1) * 128, :]
    return xt


def kernel(x, qkv_w, qkv_b, proj_w, proj_b, w3, b3, w5, b5, w7, b7, H=28, W=28):
    x = np.asarray(x, np.float32)
    qkv_b = np.asarray(qkv_b, np.float32)
    proj_b = np.asarray(proj_b, np.float32)
    assert x.shape == (B, N, C), x.shape
    assert int(H) == 28 and int(W) == 28

    shared = prep_shared(qkv_w, proj_w, w3, b3, w5, b5, w7, b7)
    has_bias = bool(np.any(qkv_b))
    nc = _get_nc(has_bias)
    if has_bias:
        shared["bq"] = np.ascontiguousarray(qkv_b[0:C].reshape(4, 128).T)
        shared["bv"] = np.ascontiguousarray(qkv_b[2 * C:3 * C].reshape(4, 128).T)
        shared["bkv"] = np.ascontiguousarray(
            qkv_b[C:3 * C].reshape(1, 2 * C)).astype(np.float16)

    in_maps = []
    for core in range(NCORES):
        m = {"xt": prep_xt(x[core * BL:(core + 1) * BL])}
        m.update(shared)
        in_maps.append(m)

    res = run_bass_kernel_spmd(nc, in_maps, list(range(NCORES)))
    global LAST_RESULT
    LAST_RESULT = res
    out = np.concatenate([r["out"][:, :N, :] for r in res.results],
                     axis=0).astype(np.float32)
    out = out + proj_b[None, None, :]
    return out.astype(np.float32)


# revision 15
# speedup vs baseline: 1.1784x; 1.1784x over previous
"""CoaT factorized-attention + CRPE block on 8 Trainium2 NeuronCores.

Sharding: pure data-parallel over batch B=32 -> 4 images per core.
Per-core layouts (all chosen so NO on-device transposes are needed):
  xt      [128, 4*NP]      feature-major input, 4 channel-tiles packed along
                           the free dim (host pre-transposes); fp16
  q       [128, 787]       feature-major, stored shifted +1 col so the image
                           tokens (1..784) sit at even fp16 offsets for DVE
  k       [T, C]           token-major fp16 (GEMM-KV: lhsT=xt tiles)
  v       [T, 4*(128+2)]   token-major fp16, per-head-pair chunk padded with
                           two 8.0 columns so the kv matmul also produces the
                           softmax denominator scaled by 1/SCALE
  kv      [c, d] per head-pair, block-diag packed 128x128, fp16; off-diag
                           blocks zeroed once, diagonals rescaled on DVE
  fa      PSUM; CRPE added into PSUM by DVE, single scalar copy to attn
  conv    feature-major on a zero-padded 34x34 fp16 image buffer; per-image
          per-channel-tile tap split across PE (fp16 diag matmuls), DVE and
          ScalarE (multiplies), tuned so all three engines stay balanced;
          the last image leans on PE (nothing else fills the pipeline tail)
  proj    out[T, 512] token-major (lhsT=attn fp16, rhs=proj_w.T fp16)
Softmax over tokens without max-subtraction (values are O(1)): ek=exp(k) on
ScalarE; 1/den and the 1/sqrt(Ch) scale fold into the kv rescale.
PSUM discipline: 4 independent tags (kv-gemm / q+v2+hp / fa+proj / conv),
every matmul group exactly 1 bank, so image b+1's GEMMs never wait on
image b's late-phase consumers. 8 banks exactly.
DMA discipline: one packed DMA per weight blob, one per image input, one per
image output (out padded to 896 rows); image-0 input issued before the bulky
conv-diag weight blob so PE starts within ~6us.
"""

import numpy as np

import concourse.bass as bass
import concourse.bacc as bacc
import concourse.mybir as mybir
import concourse.tile as tile
from concourse.bass_utils import run_bass_kernel_spmd

F32 = mybir.dt.float32
F16 = mybir.dt.float16

# taps on PE per channel-tile (bias always rides with PE when >0):
# mid-pipeline images keep PE lean; the last image fills the tail with PE.
CWD_TAPS = [9, 25, 49, 16]           # diag tiles built per ct
PE_TAPS_MID = [4, 25, 49, 4]
PE_TAPS_LAST = [9, 25, 49, 16]
DVM = 3                              # every DVM-th non-PE tap multiply -> DVE; rest ScalarE

NCORES = 8
B, N, C = 32, 785, 512
BL = B // NCORES            # 4 images per core
H = W = 28
HW = H * W                  # 784, N = 1 + HW
NH = 8                      # heads
CH = C // NH                # 64
SCALE = CH ** -0.5          # 1/8
PADW = 34                   # 28 + 2*3 (pad 3 covers 3x3/5x5/7x7 uniformly)
NP = 786                    # token columns padded even
NT = 7                      # token tiles: 6*128 + 17
TSIZES = [128, 128, 128, 128, 128, 128, 17]
TOFFS = [0, 128, 256, 384, 512, 640, 768]
NPAD = 896                  # 7*128, padded output rows per image
GS = 505                    # token split: 504 pixels = 18 image rows
VW = 130                    # v chunk width per head-pair: 128 + 2 ones cols

CT_TAPS = [(3, 1), (5, 2), (7, 3), (7, 3)]
N_DIAG = sum(t + 1 for t in CWD_TAPS)


def _tap_base(k):
    return 3 - (k // 2)


def build_conv_weights(w3, b3, w5, b5, w7, b7):
    """Per channel-tile tap weights [4][128, 49] and biases [128, 4]."""
    w3 = w3.reshape(128, 9).astype(np.float32)
    w5 = w5.reshape(192, 25).astype(np.float32)
    w7 = w7.reshape(192, 49).astype(np.float32)
    cw = np.zeros((4, 128, 49), np.float32)
    cw[0, :, :9] = w3
    cw[1, :, :25] = w5[:128]
    emb = np.zeros((64, 7, 7), np.float32)
    emb[:, 1:6, 1:6] = w5[128:192].reshape(64, 5, 5)
    cw[2, :64] = emb.reshape(64, 49)
    cw[2, 64:] = w7[:64]
    cw[3] = w7[64:192]
    cb = np.zeros((128, 4), np.float32)
    cb[:, 0] = b3
    cb[:, 1] = b5[:128]
    cb[:64, 2] = b5[128:192]
    cb[64:, 2] = b7[:64]
    cb[:, 3] = b7[64:192]
    return cw, cb


def build_cwd(cw, cb):
    """Packed diag tap weights [128, N_DIAG*128] fp16 (bias diag first)."""
    ar = np.arange(128)
    diags = []
    for ct in range(4):
        d = np.zeros((128, 128), np.float16)
        d[ar, ar] = cb[:, ct].astype(np.float16)
        diags.append(d)
        for ti in range(CWD_TAPS[ct]):
            d = np.zeros((128, 128), np.float16)
            d[ar, ar] = cw[ct][:, ti].astype(np.float16)
            diags.append(d)
    return np.stack(diags).transpose(1, 0, 2).reshape(128, -1)


def pack_rows(wT, nchunk):
    """[nchunk*128, X] -> [128, nchunk*X] partition-major packing."""
    X = wT.shape[1]
    return np.ascontiguousarray(
        wT.reshape(nchunk, 128, X).transpose(1, 0, 2).reshape(128, nchunk * X))


def build_nc(has_qkv_bias):
    nc = bacc.Bacc()

    def mm(out, lhsT, rhs, **kw):
        nc.tensor.matmul(out, lhsT, rhs, **kw)

    xt_d = nc.dram_tensor("xt", [BL, 128, 4 * NP], F16, kind="ExternalInput")
    wq_d = nc.dram_tensor("wq", [128, 4 * C], F16, kind="ExternalInput")
    wkv_d = nc.dram_tensor("wkv", [128, 4 * 2 * C], F16, kind="ExternalInput")
    wv_d = nc.dram_tensor("wv", [128, 4 * C], F16, kind="ExternalInput")
    pw_d = nc.dram_tensor("pw", [128, 4 * C], F16, kind="ExternalInput")
    cw_d = nc.dram_tensor("cw", [128, 4 * 49], F32, kind="ExternalInput")
    cb_d = nc.dram_tensor("cb", [128, 4], F32, kind="ExternalInput")
    cwd_d = nc.dram_tensor("cwd", [128, N_DIAG * 128], F16, kind="ExternalInput")
    if has_qkv_bias:
        bq_d = nc.dram_tensor("bq", [128, 4], F32, kind="ExternalInput")
        bv_d = nc.dram_tensor("bv", [128, 4], F32, kind="ExternalInput")
        bkv_d = nc.dram_tensor("bkv", [1, 2 * C], F16, kind="ExternalInput")
    out_d = nc.dram_tensor("out", [BL, NPAD, C], F16, kind="ExternalOutput")

    with tile.TileContext(nc) as tc:
        with (
            tc.tile_pool(name="const", bufs=1) as cpool,
            tc.tile_pool(name="xt", bufs=2) as xtpool,
            tc.tile_pool(name="ek", bufs=14) as ekpool,
            tc.tile_pool(name="vt", bufs=14) as vtpool,
            tc.tile_pool(name="qf", bufs=8) as qpool,
            tc.tile_pool(name="vp", bufs=2) as vppool,
            tc.tile_pool(name="ca", bufs=2) as capool,
            tc.tile_pool(name="ev", bufs=2) as evpool,
            tc.tile_pool(name="at", bufs=8) as atpool,
            tc.tile_pool(name="sm", bufs=8) as smpool,
            tc.tile_pool(name="ob", bufs=2) as obpool,
            tc.tile_pool(name="pskv", bufs=2, space="PSUM") as kvpool,
            tc.tile_pool(name="psq", bufs=2, space="PSUM") as qppool,
            tc.tile_pool(name="psfa", bufs=2, space="PSUM") as fapool,
            tc.tile_pool(name="psc", bufs=2, space="PSUM") as pscpool,
        ):
            # ---- image-0 input DMA first: PE can start within ~6us ----
            xt_bufs = []
            for i in range(2):
                t = xtpool.tile([128, 4 * NP], F16, tag="xt", name=f"xtb{i}")
                xt_bufs.append(t)
            nc.sync.dma_start(xt_bufs[0][:], xt_d[0])

            # ---- constants: one packed DMA per blob ----
            wq_t = cpool.tile([128, 4 * C], F16, tag="wq")
            wkv_t = cpool.tile([128, 8 * C], F16, tag="wkv")
            wv_t = cpool.tile([128, 4 * C], F16, tag="wv")
            pw_t = cpool.tile([128, 4 * C], F16, tag="pw")
            cw_t = cpool.tile([128, 4 * 49], F32, tag="cw")
            cb_t = cpool.tile([128, 4], F32, tag="cb")
            cwd_t = cpool.tile([128, N_DIAG * 128], F16, tag="cwd")
            nc.sync.dma_start(wkv_t[:], wkv_d[:])
            nc.sync.dma_start(wq_t[:], wq_d[:])
            nc.sync.dma_start(wv_t[:], wv_d[:])
            nc.sync.dma_start(cw_t[:], cw_d[:])
            nc.sync.dma_start(cb_t[:], cb_d[:])
            nc.sync.dma_start(pw_t[:], pw_d[:])
            nc.sync.dma_start(cwd_t[:], cwd_d[:])

            def wkv_s(kc, cols):
                return wkv_t[:, kc * 2 * C + cols.start:kc * 2 * C + cols.stop]

            def pw_s(kc):
                return pw_t[:, kc * C:(kc + 1) * C]

            def xt_s(xt, kc, cols):
                return xt[:, kc * NP + cols.start:kc * NP + cols.stop]

            def cwd_s(i):
                return cwd_t[:, i * 128:(i + 1) * 128]

            def cw_s(ct, ti):
                return cw_t[:, ct * 49 + ti:ct * 49 + ti + 1]

            onesimg = cpool.tile([128, HW], F16, tag="onesimg")
            nc.gpsimd.memset(onesimg[:], 1.0)
            if has_qkv_bias:
                bq_t = cpool.tile([128, 4], F32, tag="bq")
                bv_t = cpool.tile([128, 4], F32, tag="bv")
                bkv_t = cpool.tile([1, 2 * C], F16, tag="bkv")
                ones_t = cpool.tile([1, 128], F16, tag="ones")
                nc.sync.dma_start(bq_t[:], bq_d[:])
                nc.sync.dma_start(bv_t[:], bv_d[:])
                nc.sync.dma_start(bkv_t[:], bkv_d[:])
                nc.scalar.activation(ones_t[:], bkv_t[:, 0:128],
                    mybir.ActivationFunctionType.Identity, bias=1.0, scale=0.0)

            # ---- v tiles: denominator columns (=8.0) written ONCE ----
            vt_bufs = []
            for i in range(14):
                t = vtpool.tile([128, 4 * VW], F16, tag="vt", name=f"vt{i}")
                nc.gpsimd.memset(
                    t[:].rearrange("p (h c) -> p h c", h=4)[:, :, 128:VW], 8.0)
                vt_bufs.append(t)

            # ---- kv tiles: off-diagonal head blocks zeroed ONCE ----
            kv_bufs = []
            for i in range(8):
                t = smpool.tile([128, 128], F16, tag="kvsb", name=f"kv{i}")
                nc.gpsimd.memset(t[:], 0.0)
                kv_bufs.append(t)

            # ---- padded conv image buffers: memset borders ONCE ----
            vp_bufs, vq_bufs = [], []
            for ct in range(4):
                bs, qs = [], []
                for i in range(2):
                    t = vppool.tile([128, PADW, PADW], F16,
                                    tag=f"vp{ct}", name=f"vp{ct}_{i}")
                    nc.gpsimd.memset(t[:], 0.0)
                    bs.append(t)
                    if PE_TAPS_MID[ct] < CT_TAPS[ct][0] ** 2:
                        tq = vppool.tile([128, PADW, PADW], F16,
                                         tag=f"vq{ct}", name=f"vq{ct}_{i}")
                        nc.gpsimd.memset(tq[:], 0.0)
                        qs.append(tq)
                vp_bufs.append(bs)
                vq_bufs.append(qs)

            ob_bufs = []
            for i in range(2):
                t = obpool.tile([128, NT * C], F16, tag="ob", name=f"ob{i}")
                nc.gpsimd.memset(t[:, (NT - 1) * C:], 0.0)
                ob_bufs.append(t)

            diag_off = {}
            _o = 0
            for _ct in range(4):
                diag_off[_ct] = _o
                _o += CWD_TAPS[_ct] + 1

            for b in range(BL):
                xt_t = xt_bufs[b % 2]
                if b + 1 < BL:
                    nxt = xt_bufs[(b + 1) % 2]
                    nc.sync.dma_start(nxt[:], xt_d[b + 1])
                pe_taps = PE_TAPS_LAST if b == BL - 1 else PE_TAPS_MID

                # ---- GEMM-KV: token-major ek=exp(k) and v (+den cols) ----
                ek_t, v_t = [], []
                for tt in range(NT):
                    m = TSIZES[tt]
                    o = TOFFS[tt]
                    ek = ekpool.tile([128, C], F16, tag="ek")
                    vv = vt_bufs[(b * NT + tt) % 14]
                    for half in range(2):
                        cols = slice(512 * half, 512 * (half + 1))
                        ps = kvpool.tile([128, 512], F32, tag="pskv",
                                         name="pskv")
                        for kc in range(4):
                            mm(
                                ps[:m, :],
                                xt_s(xt_t, kc, slice(o, o + m)),
                                wkv_s(kc, cols),
                                start=(kc == 0),
                                stop=(kc == 3 and not has_qkv_bias),
                            )
                        if has_qkv_bias:
                            mm(ps[:m, :], ones_t[:, :m], bkv_t[:, cols],
                               start=False, stop=True)
                        if half == 0:
                            nc.scalar.activation(
                                ek[:m, :], ps[:m, :],
                                mybir.ActivationFunctionType.Exp)
                        else:
                            nc.scalar.copy(
                                vv[:m].rearrange(
                                    "p (h c) -> p h c", h=4)[:, :, 0:128],
                                ps[:m, :].rearrange(
                                    "p (h c) -> p h c", h=4))
                    ek_t.append(ek)
                    v_t.append(vv)

                # ---- GEMM-Q: feature-major q, stored shifted +1 col ----
                q_t = []
                for mo in range(4):
                    q = qpool.tile([128, NP + 1], F16, tag="qf")
                    for cols in (slice(0, GS), slice(GS, NP)):
                        ps = qppool.tile([128, GS], F32, tag="psq", name="psq")
                        w = cols.stop - cols.start
                        for kc in range(4):
                            mm(
                                ps[:, 0:w],
                                wq_t[:, kc * C + 128 * mo:kc * C + 128 * (mo + 1)],
                                xt_s(xt_t, kc, cols),
                                start=(kc == 0),
                                stop=(kc == 3),
                            )
                        if has_qkv_bias:
                            nc.scalar.activation(
                                q[:, 1 + cols.start:1 + cols.stop], ps[:, 0:w],
                                mybir.ActivationFunctionType.Identity,
                                bias=bq_t[:, mo:mo + 1])
                        else:
                            nc.scalar.copy(
                                q[:, 1 + cols.start:1 + cols.stop], ps[:, 0:w])
                    q_t.append(q)

                # ---- GEMM-V2: feature-major v straight into padded image ----
                vpad_t = [vp_bufs[ct][b % 2] for ct in range(4)]
                vpad1_t = [vq_bufs[ct][b % 2] if vq_bufs[ct] else None
                           for ct in range(4)]
                for ct in range(4):
                    vp = vpad_t[ct]
                    for gi, cols in enumerate((slice(0, GS), slice(GS, NP))):
                        ps = qppool.tile([128, GS], F32, tag="psq", name="psv")
                        w = cols.stop - cols.start
                        for kc in range(4):
                            mm(
                                ps[:, 0:w],
                                wv_t[:, kc * C + 128 * ct:kc * C + 128 * (ct + 1)],
                                xt_s(xt_t, kc, cols),
                                start=(kc == 0),
                                stop=(kc == 3),
                            )
                        # g0 tokens 1..504 -> rows 0..17; g1 505..784 -> 18..27
                        if gi == 0:
                            src = ps[:, 1:GS].rearrange(
                                "p (h w) -> p h w", h=18)
                            dst = vp[:, 3:21, 3:31]
                        else:
                            src = ps[:, 0:280].rearrange(
                                "p (h w) -> p h w", h=10)
                            dst = vp[:, 21:31, 3:31]
                        if has_qkv_bias:
                            nc.scalar.activation(
                                dst, src,
                                mybir.ActivationFunctionType.Identity,
                                bias=bv_t[:, ct:ct + 1])
                        else:
                            nc.scalar.copy(dst, src)
                    if vpad1_t[ct] is not None and pe_taps[ct] < CT_TAPS[ct][0] ** 2:
                        # 1-elem-shifted copy so odd-offset taps stay 4B-aligned
                        vq = vpad1_t[ct]
                        nc.vector.tensor_copy(
                            vq[:].rearrange("p a b -> p (a b)")[:, 0:1154],
                            vp[:].rearrange("p a b -> p (a b)")[:, 1:1155])

                # ---- kv per head-pair (+ den in ones cols); rescale on DVE --
                kv_t = []
                recip = smpool.tile([128, 8], F32, tag="recip", bufs=4)
                for hp in range(4):
                    cs = slice(128 * hp, 128 * (hp + 1))
                    vs = slice(VW * hp, VW * (hp + 1))
                    ps = qppool.tile([128, VW], F32, tag="psq", name="pshp")
                    for tt in range(NT):
                        m = TSIZES[tt]
                        mm(
                            ps[:], ek_t[tt][:m, cs], v_t[tt][:m, vs],
                            start=(tt == 0), stop=(tt == NT - 1))
                    nc.vector.reciprocal(
                        recip[:, 2 * hp:2 * hp + 2], ps[:, 128:VW])
                    kv = kv_bufs[(b * 4 + hp) % 8]
                    nc.vector.tensor_scalar_mul(
                        kv[0:64, 0:64], ps[0:64, 0:64],
                        recip[0:64, 2 * hp:2 * hp + 1])
                    nc.vector.tensor_scalar_mul(
                        kv[64:128, 64:128], ps[64:128, 64:128],
                        recip[64:128, 2 * hp:2 * hp + 1])
                    kv_t.append(kv)

                # ---- conv (CRPE): per-ct PE/DVE/ScalarE tap split ----
                convsrc = [None] * 4
                nmul = 0
                for ct in range(4):
                    k, _p = CT_TAPS[ct]
                    kk = k * k
                    base = _tap_base(k)
                    p = pe_taps[ct]
                    di = diag_off[ct]
                    ev = evpool.tile([128, HW], F16, tag=f"ev{ct}",
                                     name=f"ev{ct}")
                    psc_h = [None, None]
                    if p > 0:
                        for hh in range(2):
                            cols = slice(392 * hh, 392 * (hh + 1))
                            yo = 14 * hh
                            psc = pscpool.tile([128, 512], F32, tag="pscv",
                                               name="pscv")
                            if p == kk:
                                mm(psc[:, 0:392], cwd_s(di), onesimg[:, cols],
                                   start=True, stop=False)
                            for ti in range(p):
                                i, j = divmod(ti, k)
                                src = vpad_t[ct][
                                    :, base + i + yo:base + i + yo + 14,
                                    base + j:base + j + W]
                                mm(psc[:, 0:392], cwd_s(di + 1 + ti), src,
                                   start=(p < kk and ti == 0),
                                   stop=(ti == p - 1))
                            psc_h[hh] = psc
                    if p == kk:
                        for hh in range(2):
                            sl = slice(392 * hh, 392 * (hh + 1))
                            nc.vector.tensor_tensor(
                                ev[:, sl], psc_h[hh][:, 0:392],
                                q_t[ct][:, 2 + sl.start:2 + sl.stop],
                                op=mybir.AluOpType.mult)
                        convsrc[ct] = ev
                        continue

                    # DVE/ScalarE part for taps p..kk-1
                    def tap_src(ti):
                        i, j = divmod(ti, k)
                        if (base + i * PADW + base + j) % 2:
                            return vpad1_t[ct][:, base + i:base + i + H,
                                               base + j - 1:base + j - 1 + W]
                        return vpad_t[ct][:, base + i:base + i + H,
                                          base + j:base + j + W]

                    acc = capool.tile([128, H, W], F16, tag=f"ca{ct}",
                                      name=f"ca{ct}")
                    for n, ti in enumerate(range(p, kk)):
                        src = tap_src(ti)
                        if n == 0:
                            nc.scalar.activation(
                                acc[:], src,
                                mybir.ActivationFunctionType.Identity,
                                bias=cb_t[:, ct:ct + 1], scale=cw_s(ct, ti))
                        else:
                            tmp = capool.tile([128, H, W], F16, tag="tp",
                                              name="tp", bufs=4)
                            if nmul % DVM == DVM - 1:
                                nc.vector.tensor_scalar_mul(
                                    tmp[:], src, cw_s(ct, ti))
                            else:
                                nc.scalar.activation(
                                    tmp[:], src,
                                    mybir.ActivationFunctionType.Identity,
                                    scale=cw_s(ct, ti))
                            nc.vector.tensor_tensor(
                                acc[:], acc[:], tmp[:],
                                op=mybir.AluOpType.add)
                            nmul += 1
                    accf = acc[:].rearrange("p h w -> p (h w)")
                    if p > 0:
                        for hh in range(2):
                            sl = slice(392 * hh, 392 * (hh + 1))
                            nc.vector.tensor_tensor(
                                ev[:, sl], psc_h[hh][:, 0:392], accf[:, sl],
                                op=mybir.AluOpType.add)
                            nc.vector.tensor_tensor(
                                ev[:, sl], ev[:, sl],
                                q_t[ct][:, 2 + sl.start:2 + sl.stop],
                                op=mybir.AluOpType.mult)
                    else:
                        nc.vector.tensor_tensor(
                            ev[:], accf, q_t[ct][:, 2:NP],
                            op=mybir.AluOpType.mult)
                    convsrc[ct] = ev

                # ---- factor-att; CRPE added in PSUM; copies to attn ----
                # g0 = tokens 1..512 (one full psum bank, pixels 0..511);
                # cls + tokens 513..784 in g1 so proj tiles 0..3 start early.
                attn_t = []
                ob = ob_bufs[b % 2]
                for hp in range(4):
                    at = atpool.tile([128, N], F16, tag="attn")
                    ev = convsrc[hp]
                    ps = fapool.tile([128, 512], F32, tag="psfa", name="psfa")
                    mm(ps[:, 0:512], kv_t[hp][:], q_t[hp][:, 2:514],
                       start=True, stop=True)
                    nc.vector.tensor_tensor(
                        ps[:, 0:512], ps[:, 0:512], ev[:, 0:512],
                        op=mybir.AluOpType.add)
                    ps1 = fapool.tile([128, 274], F32, tag="psfa",
                                      name="psf1")
                    mm(ps1[:, 0:1], kv_t[hp][:], q_t[hp][:, 1:2],
                       start=True, stop=True)
                    nc.scalar.copy(at[:, 1:513], ps[:, 0:512])
                    nc.vector.tensor_copy(at[:, 0:1], ps1[:, 0:1])
                    mm(ps1[:, 1:274], kv_t[hp][:], q_t[hp][:, 514:787],
                       start=True, stop=True)
                    nc.vector.tensor_tensor(
                        ps1[:, 1:273], ps1[:, 1:273], ev[:, 512:HW],
                        op=mybir.AluOpType.add)
                    nc.scalar.copy(at[:, 513:N], ps1[:, 1:273])
                    attn_t.append(at)
                    if hp == 3:
                        for tt in range(4):
                            m = TSIZES[tt]
                            o = TOFFS[tt]
                            ps = fapool.tile([128, C], F32, tag="psfa",
                                             name="psout")
                            for kc in range(4):
                                mm(ps[:m, 0:C], attn_t[kc][:, o:o + m],
                                   pw_s(kc), start=(kc == 0), stop=(kc == 3))
                            nc.scalar.copy(
                                ob[:m, tt * C:(tt + 1) * C], ps[:m, 0:C])
                        nc.gpsimd.dma_start(
                            out_d[b, 0:512].rearrange("(a p) c -> p a c", p=128),
                            ob[:, 0:4 * C].rearrange("p (a c) -> p a c", a=4))

                # ---- proj tail: tiles 4..6, second half DMA ----
                for tt in range(4, NT):
                    m = TSIZES[tt]
                    o = TOFFS[tt]
                    ps = fapool.tile([128, C], F32, tag="psfa", name="psout")
                    for kc in range(4):
                        mm(
                            ps[:m, 0:C], attn_t[kc][:, o:o + m], pw_s(kc),
                            start=(kc == 0), stop=(kc == 3))
                    nc.scalar.copy(ob[:m, tt * C:(tt + 1) * C], ps[:m, 0:C])
                nc.gpsimd.dma_start(
                    out_d[b, 512:NPAD].rearrange("(a p) c -> p a c", p=128),
                    ob[:, 4 * C:].rearrange("p (a c) -> p a c", a=3))

    nc.compile()
    return nc


_NC_CACHE = {}


def _get_nc(has_qkv_bias):
    key = bool(has_qkv_bias)
    if key not in _NC_CACHE:
        _NC_CACHE[key] = build_nc(has_qkv_bias)
    return _NC_CACHE[key]


def prep_shared(qkv_w, proj_w, w3, b3, w5, b5, w7, b7):
    qkv_w = np.asarray(qkv_w, np.float32)
    proj_w = np.asarray(proj_w, np.float32)
    wqT = qkv_w[0:C].T
    wkvT = np.concatenate([qkv_w[C:2 * C].T, qkv_w[2 * C:3 * C].T], axis=1)
    wvT = qkv_w[2 * C:3 * C].T
    pwT = proj_w.T
    cw, cb = build_conv_weights(
        np.asarray(w3, np.float32), np.asarray(b3, np.float32),
        np.asarray(w5, np.float32), np.asarray(b5, np.float32),
        np.asarray(w7, np.float32), np.asarray(b7, np.float32))
    return {
        "wq": pack_rows(wqT, 4).astype(np.float16),
        "wkv": pack_rows(wkvT, 4).astype(np.float16),
        "wv": pack_rows(wvT, 4).astype(np.float16),
        "pw": pack_rows(pwT, 4).astype(np.float16),
        "cw": np.ascontiguousarray(
            cw.transpose(1, 0, 2).reshape(128, 4 * 49)),
        "cb": cb,
        "cwd": build_cwd(cw, cb),
    }


def prep_xt(xs):
    """[nb, N, C] f32 -> [nb, 128, 4*NP] f16 packed feature-major."""
    nb = xs.shape[0]
    xt = np.zeros((nb, 128, 4 * NP), np.float16)
    xsT = xs.transpose(0, 2, 1)  # [nb, C, N]
    for ct in range(4):
        xt[:, :, ct * NP:ct * NP + N] = xsT[:, ct * 128:(ct + 1) * 128, :]
    return xt


def kernel(x, qkv_w, qkv_b, proj_w, proj_b, w3, b3, w5, b5, w7, b7, H=28, W=28):
    x = np.asarray(x, np.float32)
    qkv_b = np.asarray(qkv_b, np.float32)
    proj_b = np.asarray(proj_b, np.float32)
    assert x.shape == (B, N, C), x.shape
    assert int(H) == 28 and int(W) == 28

    shared = prep_shared(qkv_w, proj_w, w3, b3, w5, b5, w7, b7)
    has_bias = bool(np.any(qkv_b))
    nc = _get_nc(has_bias)
    if has_bias:
        shared["bq"] = np.ascontiguousarray(qkv_b[0:C].reshape(4, 128).T)
        shared["bv"] = np.ascontiguousarray(qkv_b[2 * C:3 * C].reshape(4, 128).T)
        shared["bkv"] = np.ascontiguousarray(
            qkv_b[C:3 * C].reshape(1, 2 * C)).astype(np.float16)

    in_maps = []
    for core in range(NCORES):
        m = {"xt": prep_xt(x[core * BL:(core + 1) * BL])}
        m.update(shared)
        in_maps.append(m)

    res = run_bass_kernel_spmd(nc, in_maps, list(range(NCORES)))
    global LAST_RESULT
    LAST_RESULT = res
    out = np.concatenate([r["out"][:, :N, :] for r in res.results],
                     axis=0).astype(np.float32)
    out = out + proj_b[None, None, :]
    return out.astype(np.float32)
